# revision 18
# baseline (speedup 1.0000x reference)
"""Bass/Trainium2 kernel for nn_CharLevelLanguageModel (6-layer char transformer).

Strategy: data-parallel over batch (64 -> 8 cores x 8). Full forward pass in one
NEFF per core. Activations live feature-major (x_T [C, tokens]) in SBUF as
float32r. LayerNorm gains/biases are folded into the adjacent weights on the
host. On-device LN uses ones-matmul stats; rsqrt is computed as exp(-0.5*ln(v))
so every ACT function (Ln/Exp/Identity/Relu/Square) lives in ONE table set
(natural_log_exp_and_others) - no ACT table reloads. Causal masking is additive
-1e9 triangular constants accumulated into the score PSUM via identity matmuls,
so exp() gives exact zeros and no separate mask multiply is needed. GpSimd runs
ONLY PartitionBroadcast (single ucode library -> no library-reload thrash);
residual adds are fused (psum + bias + residual) in one DVE scalar_tensor_tensor.
Attention heads are software-pipelined (scores h+2 | AV h+1 | normalize h) to
keep the PE engine FIFO free of head-of-line stalls.
"""

import os
import numpy as np

import concourse.bass as bass
import concourse.mybir as mybir
import concourse.tile as tile
from concourse import bacc
from concourse.bass_utils import run_bass_kernel_spmd
from concourse.masks import make_identity

B, T, C, H, L, V = 64, 256, 384, 6, 6, 65
HS = C // H          # 64
DFF = 4 * C          # 1536
N_CORES = 8
BPC = B // N_CORES   # 8 batches per core
NTOK = BPC * T       # 2048 tokens per core
NT = NTOK // 512     # 4 column tiles of 512
KC = C // 128        # 3 feature chunks
VE = HS + 1          # v-ext row (value + ones column)
EPS = 1e-5
SCALE = HS ** -0.5
NEG = -1.0e9

f32 = mybir.dt.float32
f32r = mybir.dt.float32r
AF = mybir.ActivationFunctionType
ALU = mybir.AluOpType

N_LAYERS = int(os.environ.get("KERNEL_LAYERS", str(L)))
DEBUG = os.environ.get("KERNEL_DEBUG", "")

_cache = {}


def _build_nc():
    nc = bacc.Bacc("TRN2", target_bir_lowering=False, debug=False,
                   num_devices=N_CORES)

    x0T_d = nc.dram_tensor("x0T", [C, NTOK], f32r, kind="ExternalInput").ap()
    wqkv_d = nc.dram_tensor("wqkv", [L, C, 3 * C], f32r, kind="ExternalInput").ap()
    bqkv_d = nc.dram_tensor("bqkv", [L, 2 * C], f32, kind="ExternalInput").ap()
    wo_d = nc.dram_tensor("wo", [L, C, C], f32r, kind="ExternalInput").ap()
    bo_d = nc.dram_tensor("bo", [L, C], f32, kind="ExternalInput").ap()
    w1_d = nc.dram_tensor("w1", [L, C, DFF], f32r, kind="ExternalInput").ap()
    b1_d = nc.dram_tensor("b1", [L, DFF], f32, kind="ExternalInput").ap()
    w2_d = nc.dram_tensor("w2", [L, DFF, C], f32r, kind="ExternalInput").ap()
    b2_d = nc.dram_tensor("b2", [L, C], f32, kind="ExternalInput").ap()
    wlm_d = nc.dram_tensor("wlm", [C, V], f32r, kind="ExternalInput").ap()
    blm_d = nc.dram_tensor("blm", [V], f32, kind="ExternalInput").ap()
    outT_d = nc.dram_tensor("outT", [V, NTOK], f32, kind="ExternalOutput").ap()
    if DEBUG:
        nc.dbg_d = nc.dram_tensor("dbg", [128, 512], f32,
                                  kind="ExternalOutput").ap()

    with tile.TileContext(nc) as tc:
        _build_body(nc, tc, x0T_d, wqkv_d, bqkv_d, wo_d, bo_d, w1_d, b1_d,
                    w2_d, b2_d, wlm_d, blm_d, outT_d)
    nc.compile()
    return nc


def _build_body(nc, tc, x0T_d, wqkv_d, bqkv_d, wo_d, bo_d, w1_d, b1_d,
                w2_d, b2_d, wlm_d, blm_d, outT_d):
    import contextlib
    ctx = contextlib.ExitStack()
    p_const = ctx.enter_context(tc.tile_pool(name="consts", bufs=1))
    p_x = ctx.enter_context(tc.tile_pool(name="x", bufs=1))
    p_xn = ctx.enter_context(tc.tile_pool(name="xn", bufs=1))
    p_qk = ctx.enter_context(tc.tile_pool(name="qk", bufs=2))
    p_v = ctx.enter_context(tc.tile_pool(name="v", bufs=2))
    p_w = ctx.enter_context(tc.tile_pool(name="w", bufs=1))
    p_b = ctx.enter_context(tc.tile_pool(name="b", bufs=2))
    p_tr = ctx.enter_context(tc.tile_pool(name="tr", bufs=2))   # transient 512-wide
    p_at = ctx.enter_context(tc.tile_pool(name="at", bufs=2))   # attc accum
    p_sm = ctx.enter_context(tc.tile_pool(name="sm", bufs=3))   # rec / r_b
    p_e = ctx.enter_context(tc.tile_pool(name="e", bufs=3))
    p_st = ctx.enter_context(tc.tile_pool(name="st", bufs=2))   # LN stats rows
    p_bc = ctx.enter_context(tc.tile_pool(name="bc", bufs=2))   # LN broadcast tiles
    p_ff = ctx.enter_context(tc.tile_pool(name="ff", bufs=3))
    p_out = ctx.enter_context(tc.tile_pool(name="out", bufs=1))
    psum = ctx.enter_context(tc.tile_pool(name="psum", bufs=2, space="PSUM"))

    # ---- constants ----
    stage = p_const.tile([128, 256], f32, tag="stage")

    # M1 [128,256] additive causal mask for the kb1 half of the score tile:
    # cols 0:128 (q 0..127 vs keys 128..255) all -1e9; cols 128:256 lower-tri.
    nc.vector.memset(stage[:, 0:128], NEG)
    nc.vector.memset(stage[:, 128:256], 0.0)
    nc.gpsimd.affine_select(out=stage[:, 128:256], in_=stage[:, 128:256],
                            compare_op=ALU.is_ge, fill=NEG,
                            base=0, pattern=[[1, 128]], channel_multiplier=-1)
    M1 = p_const.tile([128, 256], f32r, tag="M1")
    nc.vector.tensor_copy(M1[:], stage[:])
    tri = M1[:, 128:256]

    stage_i = p_const.tile([128, 128], f32, tag="stage_i")
    make_identity(nc, stage_i[:])
    I128 = p_const.tile([128, 128], f32r, tag="I128")
    nc.vector.tensor_copy(I128[:], stage_i[:])

    nc.vector.memset(stage[:, 0:2], 1.0 / C)
    onesC = p_const.tile([128, 2], f32r, tag="onesC")   # 1/C for mean matmuls
    nc.vector.tensor_copy(onesC[:], stage[:, 0:2])
    nc.vector.memset(stage[:, 8:8 + H], 1.0)
    onesH = p_const.tile([128, H], f32r, tag="onesH")   # ones cols for V_ext
    nc.vector.tensor_copy(onesH[:], stage[:, 8:8 + H])
    eps_t = p_const.tile([1, 1], f32, tag="eps")
    nc.vector.memset(eps_t, EPS)

    blm_t = p_const.tile([V, 1], f32, tag="blm")
    nc.sync.dma_start(out=blm_t, in_=blm_d.rearrange("(v o) -> v o", o=1))
    wlm_t = [p_const.tile([128, V], f32r, tag=f"wlm{kc}", name=f"wlm{kc}")
             for kc in range(KC)]
    for kc in range(KC):
        nc.sync.dma_start(out=wlm_t[kc], in_=wlm_d[kc * 128:(kc + 1) * 128, :])

    # vext persistent buffers: ones column written once, values per use
    vext_t = []
    for bi in range(2):
        row = []
        for i in range(2):
            vt = p_v.tile([128, H * VE], f32r, tag=f"ve{bi}{i}",
                          name=f"ve{bi}{i}", bufs=1)
            vx = vt.rearrange("p (h e) -> p h e", h=H)
            nc.vector.tensor_copy(out=vx[:, :, HS:HS + 1], in_=onesH[:])
            row.append(vt)
        vext_t.append(row)

    def dbg_dump(ap, via_scalar=False):
        dsb = p_out.tile([128, 512], f32, tag="dbg_sb", name="dbg_sb")
        if via_scalar:
            nc.scalar.copy(dsb[0:ap.shape[0], 0:ap.shape[1]], ap)
        else:
            nc.vector.tensor_copy(dsb[0:ap.shape[0], 0:ap.shape[1]], ap)
        nc.sync.dma_start(out=nc.dbg_d, in_=dsb[:])

    # ---- residual stream ----
    x_t = [[p_x.tile([128, 512], f32r, tag=f"x{kc}_{nt}", name=f"x{kc}_{nt}")
            for nt in range(NT)] for kc in range(KC)]
    for kc in range(KC):
        for nt in range(NT):
            nc.sync.dma_start(out=x_t[kc][nt],
                              in_=x0T_d[kc * 128:(kc + 1) * 128,
                                        nt * 512:nt * 512 + 512])

    def layernorm(src_tiles, tagp):
        """xn = (x - mu) * rsqrt(var + eps), feature-major.
        Stats via ones-matmuls into one [4,512] PSUM tile per nt (rows 0:2 =
        mean, rows 2:4 = mean-square). rsqrt = exp(-0.5 * ln(var + eps)) keeps
        all ACT funcs in one table set. Broadcasts on GpSimd (its only op)."""
        out_tiles = [[p_xn.tile([128, 512], f32r, tag=f"{tagp}{kc}_{nt}",
                                name=f"{tagp}{kc}_{nt}") for nt in range(NT)]
                     for kc in range(KC)]
        sts = {}

        def stats(nt):
            mu_ps = psum.tile([2, 512], f32, tag="pa", name="mu_ps")
            sq_ps = psum.tile([2, 512], f32, tag="pc", name="sq_ps", bufs=3)
            for kc in range(KC):
                nc.tensor.matmul(mu_ps[:], onesC[:], src_tiles[kc][nt][:],
                                 start=(kc == 0), stop=(kc == KC - 1))
            for kc in range(KC):
                xsq = p_tr.tile([128, 512], f32r, tag="tr512", name="xsq")
                nc.vector.tensor_mul(xsq[:], src_tiles[kc][nt][:],
                                     src_tiles[kc][nt][:])
                nc.tensor.matmul(sq_ps[:], onesC[:], xsq[:],
                                 start=(kc == 0), stop=(kc == KC - 1))
            sts[nt] = (mu_ps, sq_ps)

        def finish(nt):
            mu_ps, sq_ps = sts[nt]
            murow = p_st.tile([1, 512], f32, tag="murow", name="murow")
            nc.scalar.copy(murow[:], mu_ps[0:1, :])
            mu2 = p_st.tile([1, 512], f32, tag="mu2", name="mu2")
            nc.vector.tensor_mul(mu2[:], murow[:], murow[:])
            varr = p_st.tile([1, 512], f32, tag="varr", name="varr")
            nc.vector.tensor_tensor(out=varr[:], in0=sq_ps[0:1, :], in1=mu2[:],
                                    op=ALU.subtract)
            lnv = p_st.tile([1, 512], f32, tag="lnv", name="lnv")
            nc.scalar.activation(lnv[:], varr[:], AF.Ln, bias=eps_t[:],
                                 scale=1.0)
            rs = p_st.tile([1, 512], f32, tag="rs", name="rs")
            nc.scalar.activation(rs[:], lnv[:], AF.Exp, bias=0.0, scale=-0.5)
            mr = p_st.tile([1, 512], f32, tag="mr", name="mr")
            nc.vector.tensor_mul(mr[:], murow[:], rs[:])
            rs_b = p_bc.tile([128, 512], f32, tag="rs_b")
            mr_b = p_bc.tile([128, 512], f32, tag="mr_b")
            nc.gpsimd.partition_broadcast(rs_b[:], rs[:])
            nc.gpsimd.partition_broadcast(mr_b[:], mr[:])
            for kc in range(KC):
                o = out_tiles[kc][nt]
                nc.vector.tensor_mul(o[:], src_tiles[kc][nt][:], rs_b[:])
                nc.vector.tensor_tensor(out=o[:], in0=o[:], in1=mr_b[:],
                                        op=ALU.subtract)

        stats(0)
        stats(1)
        for nt in range(NT):
            finish(nt)
            if nt + 2 < NT:
                stats(nt + 2)
        return out_tiles

    for l in range(N_LAYERS):
        # ---- per-layer weights ----
        wqkv_t = [p_w.tile([128, 3 * C], f32r, tag=f"wqkv{kc}", name=f"wqkv{kc}")
                  for kc in range(KC)]
        for kc in range(KC):
            nc.sync.dma_start(out=wqkv_t[kc],
                              in_=wqkv_d[l, kc * 128:(kc + 1) * 128, :])
        bqkv_t = p_b.tile([128, 6], f32, tag="bqkv")
        nc.sync.dma_start(out=bqkv_t,
                          in_=bqkv_d[l].rearrange("(a p) -> p a", p=128))
        wo_t = [p_w.tile([128, C], f32r, tag=f"wo{kc}", name=f"wo{kc}")
                for kc in range(KC)]
        for kc in range(KC):
            nc.sync.dma_start(out=wo_t[kc], in_=wo_d[l, kc * 128:(kc + 1) * 128, :])
        bo_t = p_b.tile([128, 3], f32, tag="bo")
        nc.sync.dma_start(out=bo_t, in_=bo_d[l].rearrange("(a p) -> p a", p=128))
        w1_t = [p_w.tile([128, DFF], f32r, tag=f"w1{kc}", name=f"w1{kc}")
                for kc in range(KC)]
        for kc in range(KC):
            nc.sync.dma_start(out=w1_t[kc], in_=w1_d[l, kc * 128:(kc + 1) * 128, :])
        b1_t = p_b.tile([128, 12], f32, tag="b1")
        nc.sync.dma_start(out=b1_t, in_=b1_d[l].rearrange("(a p) -> p a", p=128))
        w2_t = [p_w.tile([128, C], f32r, tag=f"w2{kc}", name=f"w2k{kc}")
                for kc in range(12)]
        for kc in range(12):
            nc.sync.dma_start(out=w2_t[kc], in_=w2_d[l, kc * 128:(kc + 1) * 128, :])
        b2_t = p_b.tile([128, 3], f32, tag="b2")
        nc.sync.dma_start(out=b2_t, in_=b2_d[l].rearrange("(a p) -> p a", p=128))

        # ---- LN1 ----
        xn = layernorm(x_t, "ln")

        # ---- attention, per pair of batches ----
        for bp in range(BPC // 2):
            nt = bp
            qk_t = [p_qk.tile([128, 512], f32r, tag=f"qk{oc}", name=f"qk{oc}")
                    for oc in range(6)]
            for oc in range(6):
                qp = psum.tile([128, 512], f32, tag="pa", name="qp")
                for kc in range(KC):
                    nc.tensor.matmul(qp[:], wqkv_t[kc][:, oc * 128:oc * 128 + 128],
                                     xn[kc][nt][:],
                                     start=(kc == 0), stop=(kc == KC - 1))
                nc.scalar.activation(qk_t[oc][:], qp[:], AF.Identity,
                                     bias=bqkv_t[:, oc:oc + 1], scale=1.0)
            attc = [p_at.tile([128, 512], f32r, tag=f"attc{kc}", name=f"attc{kc}")
                    for kc in range(KC)]
            for bi in range(2):
                q0 = bi * 256
                vext = []
                for i in range(2):
                    vp = psum.tile([128, C], f32, tag="pa", name="vp")
                    tc0 = q0 + i * 128
                    for kc in range(KC):
                        nc.tensor.matmul(vp[:], xn[kc][nt][:, tc0:tc0 + 128],
                                         wqkv_t[kc][:, 2 * C:3 * C],
                                         start=(kc == 0), stop=(kc == KC - 1))
                    vt = vext_t[bi][i]
                    vx = vt.rearrange("p (h e) -> p h e", h=H)
                    nc.vector.tensor_copy(vx[:, :, 0:HS],
                                          vp[:].rearrange("p (h d) -> p h d", h=H))
                    vext.append(vt)

                sps, aps, ems = {}, {}, {}

                def SM(h):
                    qrow = (h % 2) * 64
                    qch, kch = h // 2, 3 + h // 2
                    sp = psum.tile([128, 512], f32, tag="pc", name="sp", bufs=3)
                    qs = qk_t[qch][qrow:qrow + 64, q0:q0 + 256]
                    nc.tensor.matmul(sp[:, 0:256],
                                     qk_t[kch][qrow:qrow + 64, q0:q0 + 128],
                                     qs, start=True, stop=False,
                                     skip_group_check=True)
                    nc.tensor.matmul(sp[:, 256:512],
                                     qk_t[kch][qrow:qrow + 64, q0 + 128:q0 + 256],
                                     qs, start=False, stop=False,
                                     skip_group_check=True)
                    nc.tensor.matmul(sp[:, 0:128], I128[:], tri,
                                     start=False, stop=False,
                                     skip_group_check=True)
                    nc.tensor.matmul(sp[:, 256:512], I128[:], M1[:],
                                     start=False, stop=True,
                                     skip_group_check=True)
                    if DEBUG == "sp" and l == 0 and bp == 0 and bi == 0 and h == 0:
                        dbg_dump(sp[:], via_scalar=True)
                    e_m = p_e.tile([128, 512], f32r, tag="e", name="e_m", bufs=3)
                    nc.scalar.activation(e_m[:], sp[:], AF.Exp, bias=0.0,
                                         scale=SCALE)
                    if DEBUG == "em" and l == 0 and bp == 0 and bi == 0 and h == 0:
                        dbg_dump(e_m[:])
                    ems[h] = e_m

                def AV(h):
                    ap_ = psum.tile([VE, T], f32, tag="pd", name="ap_", bufs=3)
                    nc.tensor.matmul(ap_[:], vext[0][:, h * VE:(h + 1) * VE],
                                     ems[h][:, 0:256], start=True, stop=False)
                    nc.tensor.matmul(ap_[:], vext[1][:, h * VE:(h + 1) * VE],
                                     ems[h][:, 256:512], start=False, stop=True)
                    if DEBUG == "ap" and l == 0 and bp == 0 and bi == 0 and h == 0:
                        dbg_dump(ap_[:], via_scalar=True)
                    aps[h] = ap_

                def BN(h):
                    lnr = p_sm.tile([1, T], f32, tag="lnr", name="lnr", bufs=3)
                    nc.scalar.activation(lnr[:], aps[h][HS:HS + 1, :], AF.Ln,
                                         bias=0.0, scale=1.0)
                    rec = p_sm.tile([1, T], f32, tag="rec", name="rec", bufs=3)
                    nc.scalar.activation(rec[:], lnr[:], AF.Exp, bias=0.0,
                                         scale=-1.0)
                    r_b = p_sm.tile([64, T], f32, tag="r_b", name="r_b", bufs=3)
                    nc.gpsimd.partition_broadcast(r_b[:], rec[:])
                    if DEBUG == "rb" and l == 0 and bp == 0 and bi == 0 and h == 0:
                        dbg_dump(r_b[:])
                    qrow = (h % 2) * 64
                    nc.vector.tensor_mul(
                        attc[h // 2][qrow:qrow + 64, q0:q0 + 256],
                        aps[h][0:HS, :], r_b[:])

                for op, h in [("S", 0), ("S", 1), ("A", 0), ("S", 2), ("A", 1),
                              ("B", 0), ("S", 3), ("A", 2), ("B", 1), ("S", 4),
                              ("A", 3), ("B", 2), ("S", 5), ("A", 4), ("B", 3),
                              ("A", 5), ("B", 4), ("B", 5)]:
                    (SM if op == "S" else AV if op == "A" else BN)(h)

            if DEBUG == "attc" and l == 0 and bp == 0:
                dbg_dump(attc[0][:])
            # Wo + residual (+bo) fused on DVE
            for oc in range(KC):
                wp = psum.tile([128, 512], f32, tag="pa", name="wp")
                for kc in range(KC):
                    nc.tensor.matmul(wp[:], wo_t[kc][:, oc * 128:oc * 128 + 128],
                                     attc[kc][:], start=(kc == 0),
                                     stop=(kc == KC - 1))
                nc.vector.scalar_tensor_tensor(
                    out=x_t[oc][nt][:], in0=wp[:], scalar=bo_t[:, oc:oc + 1],
                    in1=x_t[oc][nt][:], op0=ALU.add, op1=ALU.add)

        # ---- LN2 + FFN ----
        h2 = layernorm(x_t, "ln")
        for nt in range(NT):
            fp2 = [psum.tile([128, 512], f32, tag=t, name=f"fp2{t}", bufs=bb)
                   for t, bb in (("pa", 2), ("pc", 3), ("pd", 3))]
            ffs = {}

            def FP1(k):
                fp1 = psum.tile([128, 512], f32, tag="pd", name="fp1", bufs=3)
                for kc in range(KC):
                    nc.tensor.matmul(fp1[:], w1_t[kc][:, k * 128:k * 128 + 128],
                                     h2[kc][nt][:],
                                     start=(kc == 0), stop=(kc == KC - 1))
                ff1 = p_ff.tile([128, 512], f32r, tag="ff1", name="ff1", bufs=3)
                nc.scalar.activation(ff1[:], fp1[:], AF.Relu,
                                     bias=b1_t[:, k:k + 1], scale=1.0)
                ffs[k] = ff1

            def FP2(k):
                for oc in range(KC):
                    nc.tensor.matmul(fp2[oc][:], w2_t[k][:, oc * 128:oc * 128 + 128],
                                     ffs[k][:], start=(k == 0), stop=(k == 11))

            FP1(0)
            FP1(1)
            for k in range(12):
                FP2(k)
                if k + 2 < 12:
                    FP1(k + 2)
            for oc in range(KC):
                nc.vector.scalar_tensor_tensor(
                    out=x_t[oc][nt][:], in0=fp2[oc][:], scalar=b2_t[:, oc:oc + 1],
                    in1=x_t[oc][nt][:], op0=ALU.add, op1=ALU.add)

    # ---- final LN + LM head ----
    xf = layernorm(x_t, "ln")
    for nt in range(NT):
        cols = slice(nt * 512, nt * 512 + 512)
        lp = psum.tile([V, 512], f32, tag="pa", name="lp")
        for kc in range(KC):
            nc.tensor.matmul(lp[:], wlm_t[kc][:], xf[kc][nt][:],
                             start=(kc == 0), stop=(kc == KC - 1))
        osb = p_out.tile([V, 512], f32, tag="osb")
        nc.scalar.activation(osb[:], lp[:], AF.Identity, bias=blm_t[:], scale=1.0)
        nc.sync.dma_start(out=outT_d[:, cols], in_=osb[:])

    ctx.close()


def _host_prep(inputs):
    """Fold LN affine params into weights; build per-core input maps."""
    f = lambda k: np.asarray(inputs[k], dtype=np.float32)
    idx = np.asarray(inputs["idx"]).astype(np.int64)
    tok_emb, pos_emb = f("tok_emb"), f("pos_emb")
    Wq, Wk, Wv, Wo = f("Wq"), f("Wk"), f("Wv"), f("Wo")
    bo, W1, b1, W2, b2 = f("bo"), f("W1"), f("b1"), f("W2"), f("b2")
    ln1_g, ln1_b = f("ln1_g"), f("ln1_b")
    ln2_g, ln2_b = f("ln2_g"), f("ln2_b")
    lnf_g, lnf_b = f("lnf_g"), f("lnf_b")
    Wlm, blm = f("Wlm"), f("blm")

    # [L,H,C,HS] -> [L,C,H*HS]
    Wq_all = np.transpose(Wq, (0, 2, 1, 3)).reshape(L, C, C)
    Wk_all = np.transpose(Wk, (0, 2, 1, 3)).reshape(L, C, C)
    Wv_all = np.transpose(Wv, (0, 2, 1, 3)).reshape(L, C, C)

    g1 = ln1_g[:, :, None]
    wqkv = np.concatenate([g1 * Wq_all, g1 * Wk_all, g1 * Wv_all], axis=2)
    bq = np.einsum("lc,lcd->ld", ln1_b, Wq_all)
    bk = np.einsum("lc,lcd->ld", ln1_b, Wk_all)
    bv = np.einsum("lc,lcd->ld", ln1_b, Wv_all)
    bqkv = np.concatenate([bq, bk], axis=1)
    bo2 = bo + np.einsum("ld,ldc->lc", bv, Wo)       # v-bias folds through Wo
    w1f = ln2_g[:, :, None] * W1
    b1f = b1 + np.einsum("lc,lcd->ld", ln2_b, W1)
    wlmf = lnf_g[:, None] * Wlm
    blmf = blm + lnf_b @ Wlm

    x0 = tok_emb[idx] + pos_emb[None]                # [B,T,C] f32
    in_maps = []
    for c in range(N_CORES):
        x0c = x0[c * BPC:(c + 1) * BPC].reshape(NTOK, C)
        in_maps.append({
            "x0T": np.ascontiguousarray(x0c.T),
            "wqkv": np.ascontiguousarray(wqkv),
            "bqkv": np.ascontiguousarray(bqkv),
            "wo": np.ascontiguousarray(Wo),
            "bo": np.ascontiguousarray(bo2),
            "w1": np.ascontiguousarray(w1f),
            "b1": np.ascontiguousarray(b1f),
            "w2": np.ascontiguousarray(W2),
            "b2": np.ascontiguousarray(b2),
            "wlm": np.ascontiguousarray(wlmf),
            "blm": np.ascontiguousarray(blmf),
        })
    return in_maps


def _run(inputs, trace=False):
    if "nc" not in _cache:
        _cache["nc"] = _build_nc()
    nc = _cache["nc"]
    in_maps = _host_prep(inputs)
    res = run_bass_kernel_spmd(nc, in_maps, core_ids=list(range(N_CORES)),
                               trace=trace)
    outs = []
    for c in range(N_CORES):
        outT = res.results[c]["outT"]                 # [V, NTOK]
        outs.append(outT.T.reshape(BPC, T, V))
    logits = np.concatenate(outs, axis=0).astype(np.float32)
    return logits, res


def kernel(**inputs) -> np.ndarray:
    logits, _ = _run(inputs, trace=False)
    return logits
